# revision 1
# baseline (speedup 1.0000x reference)
"""Grouped-Query Attention (S=2048, NQ=32, NKV=8, D=128, HID=4096) on 8 TRN2 NeuronCores.

Sharding: tensor-parallel over heads. Core c owns KV head c and its G=4
query heads (rows c*512..(c+1)*512 of Wq, c*128..(c+1)*128 of Wk/Wv, and
columns c*512..(c+1)*512 of Wo).  Each core computes a partial output
(row-parallel Wo); the host sums the 8 partials.

All matmuls run in bf16 (1 cycle/row on PE) with fp32 PSUM accumulation.
Layouts are chosen so no activation transpose is needed except 16 tiny
[128,128] PE transposes of vT:
  - projections produce qT/kT/vT [d, s] (d on partitions)
  - scores are computed transposed: S^T[j,i] = kT.T-slice @ qT  (lhsT=kT)
  - softmax skips max-subtraction (scores are ~N(0, 1.6^2), exp is safe)
  - rowsums come from an all-ones [128,128] stationary matmul which yields
    the denominator already broadcast across all 128 partitions
  - ctx^T[d,i] accumulates with lhsT=v[j,d]; it is exactly the lhsT layout
    the output projection needs.
"""

import os
import sys

import numpy as np
import ml_dtypes

for _p in ("/opt/trn_rl_repo", "/root/.axon_site/_ro/trn_rl_repo"):
    if os.path.isdir(_p) and _p not in sys.path:
        sys.path.insert(0, _p)

import concourse.bass as bass
import concourse.bacc as bacc
import concourse.mybir as mybir
import concourse.tile as tile
from concourse.bass_utils import run_bass_kernel_spmd
from concourse.masks import make_identity

P = 128          # partitions / head dim / PE tile
S = 2048         # sequence length
HID = 4096       # hidden dim
NCORES = 8
NH = 4           # q heads per core
DQ = NH * P      # per-core q width (512)
SC = 512         # free-dim chunk (PSUM bank = 512 fp32)
NKT = HID // P   # 32 contraction tiles over hidden
NCH = S // SC    # 4 sequence chunks
NJT = S // P     # 16 key tiles
NMT = S // P     # 16 out row tiles
NOC = HID // SC  # 8 out column chunks
SCALE = float(P) ** -0.5
BF = mybir.dt.bfloat16
F32 = mybir.dt.float32
BFNP = np.dtype(ml_dtypes.bfloat16)

_CACHE = {}


def _build():
    nc = bacc.Bacc(None, target_bir_lowering=False)
    xT = nc.declare_dram_parameter("xT", [HID, S], BF, isOutput=False)
    WqT = nc.declare_dram_parameter("WqT", [HID, DQ], BF, isOutput=False)
    WkT = nc.declare_dram_parameter("WkT", [HID, P], BF, isOutput=False)
    WvT = nc.declare_dram_parameter("WvT", [HID, P], BF, isOutput=False)
    bvp = nc.declare_dram_parameter("bvp", [P, 1], F32, isOutput=False)
    WoT = nc.declare_dram_parameter("WoT", [DQ, HID], BF, isOutput=False)
    onesd = nc.declare_dram_parameter("onesd", [P, P], mybir.dt.float32r, isOutput=False)
    out = nc.declare_dram_parameter("out", [S, HID], F32, isOutput=True)

    with tile.TileContext(nc) as tc:
        with (
            tc.tile_pool(name="consts", bufs=1) as consts,
            tc.tile_pool(name="acts", bufs=1) as acts,
            tc.tile_pool(name="xin", bufs=8) as xin,
            tc.tile_pool(name="epool", bufs=4) as epool,
            tc.tile_pool(name="rpool", bufs=2) as rpool,
            tc.tile_pool(name="opool", bufs=8) as opool,
        ):
            # ---- constants first (warmup matmuls depend only on these) ----
            ident = consts.tile([P, P], BF)
            make_identity(nc, ident)
            bv_sb = consts.tile([P, 1], F32)
            nc.sync.dma_start(out=bv_sb, in_=bvp[:, :])
            # small weights first so stage A can start early; wo (stage C) last
            wk = consts.tile([P, NKT, P], BF)
            nc.sync.dma_start(out=wk, in_=WkT[:, :].rearrange("(kt p) d -> p kt d", p=P))
            wv = consts.tile([P, NKT, P], BF)
            nc.sync.dma_start(out=wv, in_=WvT[:, :].rearrange("(kt p) d -> p kt d", p=P))
            wq = consts.tile([P, NKT, DQ], BF)
            wq_src = WqT[:, :].rearrange("(g kt p) d -> p g kt d", p=P, g=4)
            for g in range(4):
                nc.sync.dma_start(out=wq[:, g * 8:(g + 1) * 8, :], in_=wq_src[:, g])
            wo = consts.tile([P, NH, HID], BF)
            nc.sync.dma_start(out=wo, in_=WoT[:, :].rearrange("(dt p) o -> p dt o", p=P))

            onesf = consts.tile([P, P], mybir.dt.float32r)
            nc.sync.dma_start(out=onesf, in_=onesd[:, :])

            # ---- persistent activations (bf16) ----
            qT = acts.tile([P, NH, S], BF)      # per head: [128 d, 2048 s]
            kT = acts.tile([P, S], BF)          # [128 d, 2048 s]
            vT = acts.tile([P, S], BF)          # [128 d, 2048 s]
            v = acts.tile([P, NJT, P], BF)      # [128 j, jt, 128 d]
            ctxT = acts.tile([P, NH, S], BF)    # per head: [128 d, 2048 i]

            # ---- PE warmup: keep TensorE busy during initial weight DMAs so
            # the HAM clock-gate is released before real matmuls start ----
            with tc.tile_pool(name="pwarm", bufs=1, space="PSUM") as pwarm:
                wt = pwarm.tile([P, P], BF, name="warm")
                for _ in range(56):
                    nc.tensor.transpose(wt, ident, ident)

            # ---- stage A: projections (stream x once) ----
            with tc.tile_pool(name="pacc", bufs=1, space="PSUM") as pacc:
                for c in range(NCH):
                    s0 = c * SC
                    q_ps = [pacc.tile([P, SC], F32, tag="pq%d" % m, name="q_ps%d" % m)
                            for m in range(NH)]
                    k_ps = pacc.tile([P, SC], F32, tag="pk")
                    v_ps = pacc.tile([P, SC], F32, tag="pv")
                    for kt in range(NKT):
                        xt = xin.tile([P, SC], BF)
                        nc.sync.dma_start(out=xt, in_=xT[kt * P:(kt + 1) * P, s0:s0 + SC])
                        st, sp = kt == 0, kt == NKT - 1
                        nc.tensor.matmul(k_ps, lhsT=wk[:, kt, :], rhs=xt, start=st, stop=sp)
                        nc.tensor.matmul(v_ps, lhsT=wv[:, kt, :], rhs=xt, start=st, stop=sp)
                        for m in range(NH):
                            nc.tensor.matmul(q_ps[m], lhsT=wq[:, kt, m * P:(m + 1) * P],
                                             rhs=xt, start=st, stop=sp)
                    for m in range(NH):
                        nc.vector.tensor_copy(out=qT[:, m, s0:s0 + SC], in_=q_ps[m])
                    nc.vector.tensor_copy(out=kT[:, s0:s0 + SC], in_=k_ps)
                    # v = x @ Wv.T + bv  (bias is per-partition in [d, s] layout)
                    nc.scalar.activation(out=vT[:, s0:s0 + SC], in_=v_ps,
                                         func=mybir.ActivationFunctionType.Identity,
                                         bias=bv_sb, scale=1.0)
                    # v[j, d] via PE transpose, interleaved per chunk
                    for jj in range(SC // P):
                        jt = c * (SC // P) + jj
                        t_ps = pacc.tile([P, P], BF, tag="ptr", bufs=2)
                        nc.tensor.transpose(t_ps, vT[:, jt * P:(jt + 1) * P], ident)
                        nc.vector.tensor_copy(out=v[:, jt, :], in_=t_ps)

            # ---- stages B+C share one PSUM pool so they can overlap ----
            with tc.tile_pool(name="pbc", bufs=1, space="PSUM") as pbc:
                for t in range(NCH):
                    i0 = t * SC
                    # stage B: attention for query chunk t, all heads
                    for h in range(NH):
                        ctx_ps = pbc.tile([P, SC], F32, tag="pctx", bufs=2)
                        racc = rpool.tile([P, SC], mybir.dt.float32r, name="racc", bufs=2)
                        for jt in range(NJT):
                            s_ps = pbc.tile([P, SC], F32, tag="pscore", bufs=3)
                            nc.tensor.matmul(s_ps, lhsT=kT[:, jt * P:(jt + 1) * P],
                                             rhs=qT[:, h, i0:i0 + SC], start=True, stop=True)
                            e_sb = epool.tile([P, SC], BF)
                            nc.scalar.activation(out=e_sb, in_=s_ps,
                                                 func=mybir.ActivationFunctionType.Exp,
                                                 scale=SCALE)
                            st, sp = jt == 0, jt == NJT - 1
                            nc.tensor.matmul(ctx_ps, lhsT=v[:, jt, :], rhs=e_sb,
                                             start=st, stop=sp)
                            # rowsum accumulation on DVE (j is the partition dim,
                            # summed at the end with a partition-halving tree)
                            if jt == 0:
                                nc.vector.tensor_copy(out=racc, in_=e_sb)
                            else:
                                nc.vector.tensor_add(out=racc, in0=racc, in1=e_sb)
                        # cross-partition sum + broadcast in one fp32r
                        # ones-matmul (every output partition = column sum)
                        rb_ps = pbc.tile([P, SC], F32, tag="pscore", bufs=3,
                                         name="rb_ps")
                        nc.tensor.matmul(rb_ps, lhsT=onesf, rhs=racc,
                                         start=True, stop=True)
                        rbc = rpool.tile([P, SC], F32, name="rbc", bufs=2)
                        nc.vector.reciprocal(out=rbc, in_=rb_ps)
                        nc.vector.tensor_mul(out=ctxT[:, h, i0:i0 + SC],
                                             in0=ctx_ps, in1=rbc)
                    # stage C: output projection rows that chunk t completed
                    for mt in range(t * NCH, (t + 1) * NCH):
                        m0 = mt * P
                        for oc in range(NOC):
                            o0 = oc * SC
                            o_ps = pbc.tile([P, SC], F32, tag="pout", bufs=3)
                            for dt_ in range(NH):
                                nc.tensor.matmul(o_ps, lhsT=ctxT[:, dt_, m0:m0 + P],
                                                 rhs=wo[:, dt_, o0:o0 + SC],
                                                 start=dt_ == 0, stop=dt_ == NH - 1)
                            ob = opool.tile([P, SC], F32)
                            nc.vector.tensor_copy(out=ob, in_=o_ps)
                            nc.sync.dma_start(out=out[m0:m0 + P, o0:o0 + SC], in_=ob)
    nc.finalize()
    return nc


def _get_program():
    if "nc" not in _CACHE:
        _CACHE["nc"] = _build()
    return _CACHE["nc"]


def _prep_inputs(hidden_states, Wq, Wk, Wv, bv, Wo):
    x = np.asarray(hidden_states, np.float32).reshape(S, HID)
    xT = np.ascontiguousarray(x.T).astype(BFNP)
    Wq = np.asarray(Wq, np.float32)
    Wk = np.asarray(Wk, np.float32)
    Wv = np.asarray(Wv, np.float32)
    bv = np.asarray(bv, np.float32)
    Wo = np.asarray(Wo, np.float32)
    maps = []
    for c in range(NCORES):
        qs = slice(c * DQ, (c + 1) * DQ)
        ks = slice(c * P, (c + 1) * P)
        maps.append({
            "xT": xT,
            "WqT": np.ascontiguousarray(Wq[qs].T).astype(BFNP),
            "WkT": np.ascontiguousarray(Wk[ks].T).astype(BFNP),
            "WvT": np.ascontiguousarray(Wv[ks].T).astype(BFNP),
            "bvp": np.ascontiguousarray(bv[ks]).reshape(P, 1),
            "WoT": np.ascontiguousarray(Wo[:, qs].T).astype(BFNP),
            "onesd": np.ones((P, P), np.float32),
        })
    return maps


def kernel(hidden_states, Wq, Wk, Wv, bv, Wo, _trace=False, **kw):
    nc = _get_program()
    maps = _prep_inputs(hidden_states, Wq, Wk, Wv, bv, Wo)
    res = run_bass_kernel_spmd(nc, maps, list(range(NCORES)), trace=_trace, **kw)
    out = np.zeros((S, HID), np.float32)
    for c in range(NCORES):
        out += np.asarray(res.results[c]["out"], np.float32)
    if _trace:
        return out.reshape(1, S, HID), res
    return out.reshape(1, S, HID)



# revision 10
# speedup vs baseline: 1.3254x; 1.3254x over previous
"""Grouped-Query Attention (S=2048, NQ=32, NKV=8, D=128, HID=4096) on 8 TRN2 NeuronCores.

Sharding: tensor-parallel over heads. Core c owns KV head c and its G=4
query heads (rows c*512..(c+1)*512 of Wq, c*128..(c+1)*128 of Wk/Wv, and
columns c*512..(c+1)*512 of Wo).  Each core computes a partial output
(row-parallel Wo); the host sums the 8 partials.

All matmuls run in bf16 (1 cycle/row on PE) with fp32 PSUM accumulation.
The schedule is built to keep the Tensor engine continuously fed (p-state)
and to keep the DVE light (it was the bottleneck of v1):

  - stage A streams x once and computes kT/vT (all 4 chunks) plus qT for
    chunks 0-2 only; chunk 3's q projection is deferred into stage B(0)
    as PE "filler" work.
  - v[j,d] tiles come from SBUF->SBUF DMA-XBAR transposes (no PE/DVE).
  - stage B computes scores two key-tiles at a time into a 2-bank PSUM
    tile; ONE wide exp per slot halves the Scalar per-op overhead.
  - softmax row-sum accumulation runs in bf16, alternating DVE / GpSimd
    (two partial accumulators, combined by the PE ones-matmul).
  - 1/denominator uses reciprocal_approx_fast (5x faster than
    nc.vector.reciprocal; ~18 good bits).
  - stage C(t-1) output-projection matmuls are interleaved into stage
    B(t)'s slots as filler, so PE never idles while Scalar runs exps;
    their PSUM->SBUF copies run on GpSimd.
  - ctx matmuls are skewed one slot behind their scores so they never
    wait on the Scalar exp latency.
"""

import os
import sys

import numpy as np
import ml_dtypes

for _p in ("/opt/trn_rl_repo", "/root/.axon_site/_ro/trn_rl_repo"):
    if os.path.isdir(_p) and _p not in sys.path:
        sys.path.insert(0, _p)

import concourse.bass as bass
import concourse.bacc as bacc
import concourse.mybir as mybir
import concourse.tile as tile
from concourse.bass_utils import run_bass_kernel_spmd

P = 128          # partitions / head dim / PE tile
S = 2048         # sequence length
HID = 4096       # hidden dim
NCORES = 8
NH = 4           # q heads per core
DQ = NH * P      # per-core q width (512)
SC = 512         # free-dim chunk (PSUM bank = 512 fp32)
NKT = HID // P   # 32 contraction tiles over hidden
NG = 4           # kt groups (8 kt each)
KPG = NKT // NG  # kt per group (8)
NCH = S // SC    # 4 sequence chunks
NJT = S // P     # 16 key tiles
NOC = HID // SC  # 8 out column chunks
SCALE = float(P) ** -0.5
BF = mybir.dt.bfloat16
F32 = mybir.dt.float32
BFNP = np.dtype(ml_dtypes.bfloat16)

_CACHE = {}


def _build():
    nc = bacc.Bacc(None, target_bir_lowering=False)
    xT = nc.declare_dram_parameter("xT", [HID, S], BF, isOutput=False)
    WqT = nc.declare_dram_parameter("WqT", [HID, DQ], BF, isOutput=False)
    WkT = nc.declare_dram_parameter("WkT", [HID, P], BF, isOutput=False)
    WvT = nc.declare_dram_parameter("WvT", [HID, P], BF, isOutput=False)
    bvp = nc.declare_dram_parameter("bvp", [P, 1], F32, isOutput=False)
    WoT = nc.declare_dram_parameter("WoT", [DQ, HID], BF, isOutput=False)
    out = nc.declare_dram_parameter("out", [S, HID], F32, isOutput=True)

    with tile.TileContext(nc) as tc:
        with (
            tc.tile_pool(name="consts", bufs=1) as consts,
            tc.tile_pool(name="acts", bufs=1) as acts,
            tc.tile_pool(name="xin", bufs=2) as xin,
            tc.tile_pool(name="epool", bufs=4) as epool,
            tc.tile_pool(name="rpool", bufs=2) as rpool,
            tc.tile_pool(name="opool", bufs=4) as opool,
        ):
            # ---- constants ----
            ones_bf = consts.tile([P, P], BF)
            nc.vector.memset(ones_bf, 1.0)
            bv_sb = consts.tile([P, 1], F32)
            nc.sync.dma_start(out=bv_sb, in_=bvp[:, :])
            # weight-group DMAs are emitted inside chunk 0's group loop so
            # the sync-queue FIFO pipelines them with compute
            wk = consts.tile([P, NKT, P], BF)
            wv = consts.tile([P, NKT, P], BF)
            wq = consts.tile([P, NKT, DQ], BF)
            wk_src = WkT[:, :].rearrange("(kt p) d -> p kt d", p=P)
            wv_src = WvT[:, :].rearrange("(kt p) d -> p kt d", p=P)
            wq_src = WqT[:, :].rearrange("(kt p) d -> p kt d", p=P)
            wo = consts.tile([P, NH, HID], BF)
            wo_src = WoT[:, :].rearrange("(dt p) o -> p dt o", p=P)

            # ---- persistent activations (bf16) ----
            qT = acts.tile([P, NH, S], BF)      # per head: [128 d, 2048 s]
            kT = acts.tile([P, S], BF)          # [128 d, 2048 s]
            vT = acts.tile([P, S], BF)          # [128 d, 2048 s]
            v = acts.tile([P, NJT, P], BF)      # [128 j, jt, 128 d]
            ctxT = acts.tile([P, NH, S], BF)    # per head: [128 d, 2048 i]
            x3 = acts.tile([P, NKT, SC], BF)    # chunk-3 x, kept for q3 filler

            # ---- PE warmup: keep TensorE busy during initial weight DMAs so
            # the p-state ramp completes before real matmuls start ----
            with tc.tile_pool(name="pwarm", bufs=1, space="PSUM") as pwarm:
                wt = pwarm.tile([P, P], F32, name="warm")
                for _ in range(56):
                    nc.tensor.matmul(wt, lhsT=ones_bf, rhs=ones_bf,
                                     start=True, stop=True)

            # ---- stage A: projections (stream x once; q only for chunks
            # 0-2 -- chunk 3's q is deferred into stage B(0) as filler) ----
            with tc.tile_pool(name="pacc", bufs=1, space="PSUM") as pacc:
                for c in range(NCH):
                    s0 = c * SC
                    has_q = c < NCH - 1
                    k_ps = pacc.tile([P, SC], F32, tag="pk", bufs=2)
                    v_ps = pacc.tile([P, SC], F32, tag="pv", bufs=2)
                    q_ps = [pacc.tile([P, SC], F32, tag="pq%d" % m,
                                      name="q_ps%d" % m)
                            for m in range(NH)] if has_q else None
                    for g in range(NG):
                        ks = slice(g * KPG, (g + 1) * KPG)
                        if c == NCH - 1:
                            xt = x3[:, ks, :]   # DMA'd during chunk 2
                        else:
                            xt = xin.tile([P, KPG, SC], BF, name="xt")
                            nc.sync.dma_start(
                                out=xt,
                                in_=xT[g * KPG * P:(g + 1) * KPG * P,
                                       s0:s0 + SC].rearrange(
                                           "(kt p) s -> p kt s", p=P))
                            if c == 0:
                                # weight group g rides right behind its x
                                nc.sync.dma_start(out=wk[:, ks, :],
                                                  in_=wk_src[:, ks, :])
                                nc.sync.dma_start(out=wv[:, ks, :],
                                                  in_=wv_src[:, ks, :])
                                nc.sync.dma_start(out=wq[:, ks, :],
                                                  in_=wq_src[:, ks, :])
                            elif c == 1:
                                # spread wo's 4 MiB across chunk 1
                                nc.sync.dma_start(out=wo[:, g, :],
                                                  in_=wo_src[:, g, :])
                            elif c == 2:
                                # prefetch chunk-3 x behind chunk 2's x
                                nc.sync.dma_start(
                                    out=x3[:, ks, :],
                                    in_=xT[g * KPG * P:(g + 1) * KPG * P,
                                           3 * SC:4 * SC].rearrange(
                                               "(kt p) s -> p kt s", p=P))
                        for kk in range(KPG):
                            kt = g * KPG + kk
                            st, sp = kt == 0, kt == NKT - 1
                            nc.tensor.matmul(k_ps, lhsT=wk[:, kt, :],
                                             rhs=xt[:, kk, :], start=st, stop=sp)
                        for kk in range(KPG):
                            kt = g * KPG + kk
                            st, sp = kt == 0, kt == NKT - 1
                            nc.tensor.matmul(v_ps, lhsT=wv[:, kt, :],
                                             rhs=xt[:, kk, :], start=st, stop=sp)
                        if has_q:
                            for m in range(NH):
                                for kk in range(KPG):
                                    kt = g * KPG + kk
                                    st, sp = kt == 0, kt == NKT - 1
                                    nc.tensor.matmul(
                                        q_ps[m],
                                        lhsT=wq[:, kt, m * P:(m + 1) * P],
                                        rhs=xt[:, kk, :], start=st, stop=sp)
                    nc.vector.tensor_copy(out=kT[:, s0:s0 + SC], in_=k_ps)
                    # v = x @ Wv.T + bv  (bias is per-partition in [d, s])
                    nc.scalar.activation(out=vT[:, s0:s0 + SC], in_=v_ps,
                                         func=mybir.ActivationFunctionType.Identity,
                                         bias=bv_sb, scale=1.0)
                    if has_q:
                        for m in range(NH):
                            nc.vector.tensor_copy(out=qT[:, m, s0:s0 + SC],
                                                  in_=q_ps[m])
                    # v[j, d] via DMA-XBAR transpose (no PE/DVE work)
                    for jj in range(SC // P):
                        jt = c * (SC // P) + jj
                        nc.sync.dma_start(out=v[:, jt, :],
                                          in_=vT[:, jt * P:(jt + 1) * P],
                                          transpose=True)

            # ---- stages B+C: attention with interleaved filler ----
            # B(t) slots: 4 heads x 8 wide (2-key-tile) slots = 32 slots.
            # Filler per slot: t==0 -> 4 q3-projection matmuls;
            #                  t>=1 -> one C(t-1) group (4 matmuls + copy).
            NSL = NJT // 2  # 8 wide slots per head
            with tc.tile_pool(name="pbc", bufs=1, space="PSUM") as pbc:
                for t in range(NCH):
                    i0 = t * SC
                    for h in range(NH):
                        ctx_ps = pbc.tile([P, SC], F32, tag="pctx", bufs=2)
                        racc_d = rpool.tile([P, SC], BF, tag="racc_d",
                                            name="racc_d")
                        racc_g = rpool.tile([P, SC], BF, tag="racc_g",
                                            name="racc_g")
                        pend_ctx = None  # skewed: ctx for previous slot
                        q3_ps = None
                        if t == 0:
                            q3_ps = pbc.tile([P, SC], F32, tag="pfill",
                                             bufs=2, name="q3_ps")
                        for jp in range(NSL):
                            jt0, jt1 = 2 * jp, 2 * jp + 1
                            s_wide = pbc.tile([P, 2 * SC], F32, tag="psw",
                                              bufs=2, name="s_wide")
                            nc.tensor.matmul(s_wide[:, :SC],
                                             lhsT=kT[:, jt0 * P:(jt0 + 1) * P],
                                             rhs=qT[:, h, i0:i0 + SC],
                                             start=True, stop=True)
                            nc.tensor.matmul(s_wide[:, SC:],
                                             lhsT=kT[:, jt1 * P:(jt1 + 1) * P],
                                             rhs=qT[:, h, i0:i0 + SC],
                                             start=True, stop=True)
                            e_wide = epool.tile([P, 2 * SC], BF, name="e_wide")
                            nc.scalar.activation(
                                out=e_wide, in_=s_wide,
                                func=mybir.ActivationFunctionType.Exp,
                                scale=SCALE)
                            # ---- filler matmuls (keep PE busy while exp runs)
                            if t == 0:
                                for kk in range(4):
                                    kt = jp * 4 + kk
                                    nc.tensor.matmul(
                                        q3_ps,
                                        lhsT=wq[:, kt, h * P:(h + 1) * P],
                                        rhs=x3[:, kt, :],
                                        start=kt == 0, stop=kt == NKT - 1)
                            else:
                                mt = 4 * (t - 1) + h
                                oc = jp
                                m0, o0 = mt * P, oc * SC
                                o_ps = pbc.tile([P, SC], F32, tag="pfill",
                                                bufs=2, name="o_ps")
                                for dt_ in range(NH):
                                    nc.tensor.matmul(
                                        o_ps,
                                        lhsT=ctxT[:, dt_, m0:m0 + P],
                                        rhs=wo[:, dt_, o0:o0 + SC],
                                        start=dt_ == 0, stop=dt_ == NH - 1)
                                ob = opool.tile([P, SC], F32, name="ob")
                                nc.vector.tensor_copy(out=ob, in_=o_ps)
                                nc.sync.dma_start(out=out[m0:m0 + P,
                                                          o0:o0 + SC], in_=ob)
                            # ---- skewed ctx matmuls (previous slot's exp)
                            if pend_ctx is not None:
                                pe, pj0, pj1 = pend_ctx
                                nc.tensor.matmul(ctx_ps, lhsT=v[:, pj0, :],
                                                 rhs=pe[:, :SC],
                                                 start=pj0 == 0, stop=False)
                                nc.tensor.matmul(ctx_ps, lhsT=v[:, pj1, :],
                                                 rhs=pe[:, SC:],
                                                 start=False, stop=False)
                            pend_ctx = (e_wide, jt0, jt1)
                            # ---- row-sum accumulation: alternate DVE/GpSimd
                            eng = nc.vector if jp % 2 == 0 else nc.gpsimd
                            acc = racc_d if jp % 2 == 0 else racc_g
                            if jp < 2:
                                eng.tensor_copy(out=acc, in_=e_wide[:, :SC])
                            else:
                                eng.tensor_add(out=acc, in0=acc,
                                               in1=e_wide[:, :SC])
                            eng.tensor_add(out=acc, in0=acc, in1=e_wide[:, SC:])
                        # q3 copy first so the next head's filler unblocks
                        if t == 0:
                            nc.vector.tensor_copy(
                                out=qT[:, h, 3 * SC:4 * SC], in_=q3_ps)
                        # drain the skewed ctx
                        pe, pj0, pj1 = pend_ctx
                        nc.tensor.matmul(ctx_ps, lhsT=v[:, pj0, :],
                                         rhs=pe[:, :SC], start=False, stop=False)
                        nc.tensor.matmul(ctx_ps, lhsT=v[:, pj1, :],
                                         rhs=pe[:, SC:], start=False, stop=True)
                        # denominator: cross-partition sum + broadcast via
                        # ones-matmul over both partial accumulators
                        rb_ps = pbc.tile([P, SC], F32, tag="pfill", bufs=2,
                                         name="rb_ps")
                        nc.tensor.matmul(rb_ps, lhsT=ones_bf, rhs=racc_d,
                                         start=True, stop=False)
                        nc.tensor.matmul(rb_ps, lhsT=ones_bf, rhs=racc_g,
                                         start=False, stop=True)
                        rbc = rpool.tile([P, SC], F32, tag="rbc", name="rbc")
                        nc.vector.reciprocal_approx_fast(out=rbc, in_=rb_ps)
                        nc.vector.tensor_mul(out=ctxT[:, h, i0:i0 + SC],
                                             in0=ctx_ps, in1=rbc)
                # ---- C(3) tail: last chunk's output projection ----
                for h in range(NH):
                    for oc in range(NOC):
                        mt = 4 * (NCH - 1) + h
                        m0, o0 = mt * P, oc * SC
                        o_ps = pbc.tile([P, SC], F32, tag="pfill", bufs=2,
                                        name="o_ps")
                        for dt_ in range(NH):
                            nc.tensor.matmul(o_ps,
                                             lhsT=ctxT[:, dt_, m0:m0 + P],
                                             rhs=wo[:, dt_, o0:o0 + SC],
                                             start=dt_ == 0, stop=dt_ == NH - 1)
                        ob = opool.tile([P, SC], F32, name="ob")
                        nc.vector.tensor_copy(out=ob, in_=o_ps)
                        nc.sync.dma_start(out=out[m0:m0 + P, o0:o0 + SC],
                                          in_=ob)
    nc.finalize()
    return nc


def _get_program():
    if "nc" not in _CACHE:
        _CACHE["nc"] = _build()
    return _CACHE["nc"]


def _prep_inputs(hidden_states, Wq, Wk, Wv, bv, Wo):
    x = np.asarray(hidden_states, np.float32).reshape(S, HID)
    xT = np.ascontiguousarray(x.T).astype(BFNP)
    Wq = np.asarray(Wq, np.float32)
    Wk = np.asarray(Wk, np.float32)
    Wv = np.asarray(Wv, np.float32)
    bv = np.asarray(bv, np.float32)
    Wo = np.asarray(Wo, np.float32)
    maps = []
    for c in range(NCORES):
        qs = slice(c * DQ, (c + 1) * DQ)
        ks = slice(c * P, (c + 1) * P)
        maps.append({
            "xT": xT,
            "WqT": np.ascontiguousarray(Wq[qs].T).astype(BFNP),
            "WkT": np.ascontiguousarray(Wk[ks].T).astype(BFNP),
            "WvT": np.ascontiguousarray(Wv[ks].T).astype(BFNP),
            "bvp": np.ascontiguousarray(bv[ks]).reshape(P, 1),
            "WoT": np.ascontiguousarray(Wo[:, qs].T).astype(BFNP),
        })
    return maps


def kernel(hidden_states, Wq, Wk, Wv, bv, Wo, _trace=False, **kw):
    nc = _get_program()
    maps = _prep_inputs(hidden_states, Wq, Wk, Wv, bv, Wo)
    res = run_bass_kernel_spmd(nc, maps, list(range(NCORES)), trace=_trace, **kw)
    out = np.zeros((S, HID), np.float32)
    for c in range(NCORES):
        out += np.asarray(res.results[c]["out"], np.float32)
    if _trace:
        return out.reshape(1, S, HID), res
    return out.reshape(1, S, HID)
